# revision 1
# baseline (speedup 1.0000x reference)
"""Trainium2 Bass kernel for loss = sum((X[:,None]*A - I)**2), N=8192.

Algebraic decomposition (avoids materializing the residual):
    loss = sum_ij (x_i*a_ij)^2  -  2*sum_i x_i*a_ii  +  N
         = sum_i x_i^2 * r_i    -  2*sum_i x_i*d_i   +  N
where r_i = sum_j a_ij^2 (row sums of squares) and d_i = a_ii.

Sharding: A row-wise across 8 cores (1024 rows each). Each core streams its
32 MB shard from HBM once in [128, 8192] tiles; ScalarE's fused
activation(Square, accum_out) computes per-row sums of squares in a single
pass per tile (~7 us/tile, well under the ~12 us/tile DMA floor, so the
kernel stays memory-bound). A short VectorE epilogue folds in x and the
diagonal, GPSIMD reduces across partitions, and the host sums the 8 scalar
partials (+N) in float64.
"""

import numpy as np

import concourse.bacc as bacc
import concourse.mybir as mybir
from concourse.tile import TileContext
from concourse.bass_utils import run_bass_kernel_spmd

N = 8192
NCORES = 8
ROWS = N // NCORES  # 1024 rows per core
P = 128  # SBUF partitions
TILES = ROWS // P  # 8 row-tiles of 128 rows per core
F = N  # full-row chunk: [128, 8192] f32 = 4 MiB per DMA

_DT = mybir.dt.float32


def build_nc(reps=1):
    """reps>1 repeats the whole per-core computation in one NEFF; used by
    the timing harness to measure per-iteration device time by slope."""
    nc = bacc.Bacc("TRN2", target_bir_lowering=False)

    a_shard = nc.dram_tensor("a_shard", [ROWS, N], _DT, kind="ExternalInput")
    x_shard = nc.dram_tensor("x_shard", [P, TILES], _DT, kind="ExternalInput")
    d_shard = nc.dram_tensor("d_shard", [P, TILES], _DT, kind="ExternalInput")
    out = nc.dram_tensor("out", [P, reps], _DT, kind="ExternalOutput")

    a_tiles = a_shard.rearrange("(t p) n -> t p n", p=P)

    with TileContext(nc) as tc:
        with (
            tc.tile_pool(name="a", bufs=4) as apool,
            tc.tile_pool(name="small", bufs=1) as small,
        ):
            racc = small.tile([P, TILES], _DT, tag="racc")
            xst = small.tile([P, TILES], _DT, tag="xs")
            dst = small.tile([P, TILES], _DT, tag="ds")
            nc.sync.dma_start(out=xst[:], in_=x_shard[:])
            nc.sync.dma_start(out=dst[:], in_=d_shard[:])

            # Throwaway full-size output for the fused square+reduce:
            # stride-0 broadcast of a [P,1] tile, so no [P,F] scratch is
            # needed (qr.py's safe_norm trick).
            dummy = small.tile([P, 1], _DT, tag="dummy")

            for _rep in range(reps):
                for t in range(TILES):
                    at = apool.tile([P, F], _DT, tag="a")
                    nc.sync.dma_start(out=at[:], in_=a_tiles[t])
                    nc.scalar.activation(
                        out=dummy.broadcast_to(at.shape),
                        in_=at[:],
                        func=mybir.ActivationFunctionType.Square,
                        accum_out=racc[:, t : t + 1],
                    )

                # partial = sum_{p,t} x*(r*x - 2*d)
                t1 = small.tile([P, TILES], _DT, tag="t1")
                nc.vector.tensor_mul(out=t1[:], in0=racc[:], in1=xst[:])
                t2 = small.tile([P, TILES], _DT, tag="t2")
                nc.vector.scalar_tensor_tensor(
                    out=t2[:],
                    in0=dst[:],
                    scalar=-2.0,
                    in1=t1[:],
                    op0=mybir.AluOpType.mult,
                    op1=mybir.AluOpType.add,
                )
                t3 = small.tile([P, TILES], _DT, tag="t3")
                nc.vector.tensor_mul(out=t3[:], in0=t2[:], in1=xst[:])
                comb = small.tile([P, 1], _DT, tag="comb")
                nc.vector.reduce_sum(comb[:], t3[:], axis=mybir.AxisListType.X)
                # Ship the [128,1] per-partition partials; the host does the
                # final 1024-value sum in float64 (better precision than a
                # sequential fp32 partition reduce of ~65K-magnitude terms).
                nc.sync.dma_start(out=out[:, _rep : _rep + 1], in_=comb[:])

    nc.compile()
    return nc


_nc_cache = {}


def _get_nc(reps=1):
    if reps not in _nc_cache:
        _nc_cache[reps] = build_nc(reps)
    return _nc_cache[reps]


def _shard_inputs(X, A):
    X = np.ascontiguousarray(np.asarray(X, dtype=np.float32))
    A = np.ascontiguousarray(np.asarray(A, dtype=np.float32))
    d = np.ascontiguousarray(A.diagonal()).astype(np.float32)
    in_maps = []
    for c in range(NCORES):
        r0 = c * ROWS
        in_maps.append(
            {
                "a_shard": A[r0 : r0 + ROWS],
                "x_shard": np.ascontiguousarray(
                    X[r0 : r0 + ROWS].reshape(TILES, P).T
                ),
                "d_shard": np.ascontiguousarray(
                    d[r0 : r0 + ROWS].reshape(TILES, P).T
                ),
            }
        )
    return in_maps


def _run(inputs, trace=False):
    nc = _get_nc()
    in_maps = _shard_inputs(inputs["X"], inputs["A"])
    res = run_bass_kernel_spmd(
        nc, in_maps, core_ids=list(range(NCORES)), trace=trace
    )
    partials = np.array(
        [r["out"][:, 0].astype(np.float64).sum() for r in res.results],
        dtype=np.float64,
    )
    total = np.float32(partials.sum() + float(N))
    return np.array(total, dtype=np.float32), res


def kernel(**inputs):
    out, _ = _run(inputs, trace=False)
    return out



# revision 2
# speedup vs baseline: 1.2227x; 1.2227x over previous
"""Trainium2 Bass kernel for loss = sum((X[:,None]*A - I)**2), N=8192.

Algebraic decomposition (avoids materializing the residual):
    loss = sum_ij (x_i*a_ij)^2  -  2*sum_i x_i*a_ii  +  N
         = sum_i x_i^2 * r_i    -  2*sum_i x_i*d_i   +  N
where r_i = sum_j a_ij^2 (row sums of squares) and d_i = a_ii.

Sharding: A row-wise across 8 cores (1024 rows each). Each core streams its
32 MB shard from HBM once in [128, 2048] (1 MiB) chunks; ScalarE's fused
activation(Square, accum_out) computes per-row partial sums of squares in a
single pass per chunk (~2.3 us/chunk vs ~2.9 us/chunk DMA, so the kernel
stays memory-bound). The x/diag fold is precomputed on the host as two
[128, 32] constant tensors so the device epilogue is just
    y = racc * x2 + c;  comb = rowsum(y)  ->  [128, 1]
followed by a ones-vector matmul on the (otherwise idle) TensorE to reduce
across partitions to a single [1, 1] scalar. That keeps the output DMA to
ONE descriptor: a [128, 1] output would fan into 128 4-byte descriptors
whose serialized HBM write receipts cost ~10 us at kernel tail (measured).
The host sums the 8 per-core scalars (+N) in float64.
"""

import numpy as np

import concourse.bacc as bacc
import concourse.mybir as mybir
from concourse.tile import TileContext
from concourse.bass_utils import run_bass_kernel_spmd

N = 8192
NCORES = 8
ROWS = N // NCORES  # 1024 rows per core
P = 128  # SBUF partitions
TILES = ROWS // P  # 8 row-tiles of 128 rows per core
CPT = 4  # chunks per row-tile
CHUNK = N // CPT  # 2048 columns -> [128, 2048] f32 = 1 MiB per DMA
NCHUNK = TILES * CPT  # 32 chunks per core

_DT = mybir.dt.float32


def build_nc():
    nc = bacc.Bacc("TRN2", target_bir_lowering=False)

    a_shard = nc.dram_tensor("a_shard", [ROWS, N], _DT, kind="ExternalInput")
    # Host-precomputed epilogue constants, both [P, NCHUNK]:
    #   x2c[p, t*CPT+c] = X[row]**2          (row = shard row t*128+p)
    #   cc [p, t*CPT+c] = -2*X[row]*d[row]/CPT
    # so that sum_c (racc*x2c + cc)[p, t*CPT+c] = x^2*r - 2*x*d for that row.
    x2c = nc.dram_tensor("x2c", [P, NCHUNK], _DT, kind="ExternalInput")
    cc = nc.dram_tensor("cc", [P, NCHUNK], _DT, kind="ExternalInput")
    out = nc.dram_tensor("out", [1, 1], _DT, kind="ExternalOutput")

    a_tiles = a_shard.rearrange("(t p) n -> t p n", p=P)

    with TileContext(nc) as tc:
        with (
            tc.tile_pool(name="a", bufs=12) as apool,
            tc.tile_pool(name="small", bufs=1) as small,
            tc.tile_pool(name="ps", bufs=1, space="PSUM") as pspool,
        ):
            racc = small.tile([P, NCHUNK], _DT, tag="racc")
            x2t = small.tile([P, NCHUNK], _DT, tag="x2")
            ct = small.tile([P, NCHUNK], _DT, tag="c")
            ones = small.tile([P, 1], _DT, tag="ones")
            # Constant loads ride the ACT HWDGE ring so the SP ring carries
            # nothing but the A stream; ones comes from GpSimd. None of this
            # delays the first A-chunk DMA issue on Sync.
            nc.scalar.dma_start(out=x2t[:], in_=x2c[:])
            nc.scalar.dma_start(out=ct[:], in_=cc[:])
            nc.gpsimd.memset(ones[:], 1.0)

            # Throwaway full-size output for the fused square+reduce:
            # stride-0 broadcast of a [P,1] tile, so no [P,CHUNK] scratch is
            # needed (qr.py's safe_norm trick).
            dummy = small.tile([P, 1], _DT, tag="dummy")

            for t in range(TILES):
                for c in range(CPT):
                    k = t * CPT + c
                    at = apool.tile([P, CHUNK], _DT, tag="a")
                    nc.sync.dma_start(
                        out=at[:], in_=a_tiles[t][:, c * CHUNK : (c + 1) * CHUNK]
                    )
                    nc.scalar.activation(
                        out=dummy.broadcast_to(at.shape),
                        in_=at[:],
                        func=mybir.ActivationFunctionType.Square,
                        accum_out=racc[:, k : k + 1],
                    )

            # Epilogue: per-partition partials, then cross-partition reduce
            # on TensorE (ones^T @ comb) so the output DMA is 1 descriptor.
            y = small.tile([P, NCHUNK], _DT, tag="y")
            nc.vector.tensor_mul(out=y[:], in0=racc[:], in1=x2t[:])
            y2 = small.tile([P, NCHUNK], _DT, tag="y2")
            nc.vector.tensor_add(out=y2[:], in0=y[:], in1=ct[:])
            comb = small.tile([P, 1], _DT, tag="comb")
            nc.vector.reduce_sum(comb[:], y2[:], axis=mybir.AxisListType.X)
            ps = pspool.tile([1, 1], _DT, tag="ps")
            nc.tensor.matmul(ps[:], ones[:], comb[:], start=True, stop=True)
            res = small.tile([1, 1], _DT, tag="res")
            nc.vector.tensor_copy(res[:], ps[:])
            nc.sync.dma_start(out=out[:], in_=res[:])

    nc.compile()
    return nc


_nc_cache = {}


def _get_nc():
    if "nc" not in _nc_cache:
        _nc_cache["nc"] = build_nc()
    return _nc_cache["nc"]


def _shard_inputs(X, A):
    X = np.ascontiguousarray(np.asarray(X, dtype=np.float32))
    A = np.ascontiguousarray(np.asarray(A, dtype=np.float32))
    d = np.ascontiguousarray(A.diagonal()).astype(np.float32)
    in_maps = []
    for core in range(NCORES):
        r0 = core * ROWS
        xs = X[r0 : r0 + ROWS].reshape(TILES, P).T  # [P, TILES]
        ds = d[r0 : r0 + ROWS].reshape(TILES, P).T
        x2 = np.repeat(xs * xs, CPT, axis=1)  # [P, NCHUNK]
        cc = np.repeat(-2.0 * xs * ds / CPT, CPT, axis=1)
        in_maps.append(
            {
                "a_shard": A[r0 : r0 + ROWS],
                "x2c": np.ascontiguousarray(x2.astype(np.float32)),
                "cc": np.ascontiguousarray(cc.astype(np.float32)),
            }
        )
    return in_maps


def _run(inputs, trace=False):
    nc = _get_nc()
    in_maps = _shard_inputs(inputs["X"], inputs["A"])
    res = run_bass_kernel_spmd(
        nc, in_maps, core_ids=list(range(NCORES)), trace=trace
    )
    partials = np.array(
        [float(r["out"][0, 0]) for r in res.results], dtype=np.float64
    )
    total = np.float32(partials.sum() + float(N))
    return np.array(total, dtype=np.float32), res


def kernel(**inputs):
    out, _ = _run(inputs, trace=False)
    return out
